# revision 2
# baseline (speedup 1.0000x reference)
"""NeuralPonds MoE-routing gather kernel for 8 Trainium2 NeuronCores. v2.3.

Computation (matches the reference up to fp16 rounding of the output):
    flavor[b,s] = int(abs(sum_d context[b,s,d])) % 10000
    out[b,s,:]  = tables[pond[b,s], flavor[b,s], :]

Data-parallel over tokens (2048/core), tables replicated. Per core
(~21 MB DMA payload, ~0.38 GB/us aggregate engine ceiling):
  - ALL ctx column loads queued up front on both HWDGE rings (8 MB) so
    the DMA engines saturate from t=0 while the SWDGE queue spins up,
  - per 2-column chunk: DVE reduce + 5-op index math
    (idx = int32(|sum| - 0.5) + pond*10000, rtn cast -> floor),
  - per column: SWDGE indirect gather of f32 table rows (8 MB),
    ACT f32->f16 cast, f16 store on the HWDGE rings (4 MB).
Host upcasts the f16 output to f32 (tolerance 2e-2 >> f16's 2e-4).
"""

import os

import numpy as np

import concourse.bass as bass
import concourse.tile as tile
from concourse import bacc, mybir
from concourse import bass_utils

P = 128            # SBUF partitions
D = 1024           # d_model
N_CORES = 8
TOK_PER_CORE = 2048
NCOL = TOK_PER_CORE // P   # 16 token-columns per core
K = 2                      # columns per reduce chunk
N_ROWS = 100000            # 10 ponds x 10000 capacity
POND_MOD = 10000

f32 = mybir.dt.float32
f16 = mybir.dt.float16
i32 = mybir.dt.int32


def build_nc():
    nc = bacc.Bacc(
        "TRN2",
        target_bir_lowering=False,
        debug=False,
        enable_asserts=False,
        num_devices=N_CORES,
    )
    ctx = nc.dram_tensor("ctx", [TOK_PER_CORE, D], f32, kind="ExternalInput").ap()
    ponds = nc.dram_tensor("ponds", [TOK_PER_CORE], i32, kind="ExternalInput").ap()
    tables = nc.dram_tensor("tables", [N_ROWS, D], f32, kind="ExternalInput").ap()
    out = nc.dram_tensor("out", [TOK_PER_CORE, D], f16, kind="ExternalOutput").ap()

    # token t = p*NCOL + n  ->  partition p, column n (contiguous per partition)
    ctx_r = ctx.rearrange("(p n) m -> p n m", p=P)      # [128, 16, 1024]
    out_r = out.rearrange("(p n) m -> p n m", p=P)      # [128, 16, 1024]
    ponds_r = ponds.rearrange("(p n) -> p n", p=P)      # [128, 16]
    tables_nar = tables.rearrange("r (a m) -> (r a) m", m=16)  # warmup view

    with tile.TileContext(nc) as tc:
        from contextlib import ExitStack

        with ExitStack() as es:
            const = es.enter_context(tc.tile_pool(name="const", bufs=1))
            cpool = es.enter_context(tc.tile_pool(name="ctxp", bufs=NCOL // K))
            spool = es.enter_context(tc.tile_pool(name="small", bufs=4))
            gpool = es.enter_context(tc.tile_pool(name="gath", bufs=8))
            hpool = es.enter_context(tc.tile_pool(name="half", bufs=8))

            # SWDGE warmup: no-dependency indirect gather (8 KB) so the slow
            # qPoolDynamic spin-up overlaps the ctx loads.
            warm_idx = const.tile([P, 1], i32)
            nc.vector.memset(warm_idx[:], 0)
            warm_g = const.tile([P, 16], f32)
            nc.gpsimd.indirect_dma_start(
                out=warm_g[:],
                out_offset=None,
                in_=tables_nar,
                in_offset=bass.IndirectOffsetOnAxis(ap=warm_idx[:, 0:1], axis=0),
            )

            # ponds first on ring A (tiny, unblocks index math immediately)
            ponds_t = const.tile([P, NCOL], i32)
            nc.sync.dma_start(out=ponds_t[:], in_=ponds_r)
            pondx = const.tile([P, NCOL], i32)
            nc.vector.tensor_scalar_mul(pondx[:], ponds_t[:], POND_MOD)

            # queue ALL ctx loads up front, alternating rings
            ctiles = []
            for c in range(NCOL // K):
                ctile = cpool.tile([P, K, D], f32, tag="c")
                load_eng = nc.sync if c % 2 == 0 else nc.scalar
                load_eng.dma_start(out=ctile[:], in_=ctx_r[:, c * K:(c + 1) * K, :])
                ctiles.append(ctile)

            for c in range(NCOL // K):
                cols = slice(c * K, (c + 1) * K)
                sums = spool.tile([P, K], f32)
                nc.vector.tensor_reduce(
                    out=sums[:], in_=ctiles[c][:],
                    axis=mybir.AxisListType.X, op=mybir.AluOpType.add,
                )
                # x = |sums| via sign-bit clear (one DVE op)
                x = spool.tile([P, K], f32)
                nc.vector.tensor_scalar(
                    out=x[:].bitcast(i32), in0=sums[:].bitcast(i32),
                    scalar1=0x7FFFFFFF, scalar2=None,
                    op0=mybir.AluOpType.bitwise_and,
                )
                # floor via x-0.5 then round-to-nearest f32->i32 cast (HW rtn;
                # |sums| is never within 2e-5 of an integer so the 0.5 shift
                # is exact at these magnitudes)
                nc.vector.tensor_scalar_sub(x[:], x[:], 0.5)
                flav = spool.tile([P, K], i32)
                nc.vector.tensor_copy(out=flav[:], in_=x[:])
                # idx = pond*10000 + flavor, clamped (i32, exact)
                idx = spool.tile([P, K], i32)
                nc.vector.tensor_tensor(
                    out=idx[:], in0=flav[:], in1=pondx[:, cols],
                    op=mybir.AluOpType.add,
                )
                nc.vector.tensor_scalar_min(idx[:], idx[:], N_ROWS - 1)

                for j in range(K):
                    n = c * K + j
                    g = gpool.tile([P, D], f32, tag="g")
                    nc.gpsimd.indirect_dma_start(
                        out=g[:],
                        out_offset=None,
                        in_=tables,
                        in_offset=bass.IndirectOffsetOnAxis(ap=idx[:, j:j + 1], axis=0),
                    )
                    h = hpool.tile([P, D], f16, tag="h")
                    nc.scalar.copy(out=h[:], in_=g[:])  # f32 -> f16 on ACT
                    store_eng = nc.sync if n % 2 == 0 else nc.scalar
                    store_eng.dma_start(out=out_r[:, n, :], in_=h[:])

    nc.compile()
    return nc


_NC = None
LAST_RESULTS = None


def _get_nc():
    global _NC
    if _NC is None:
        _NC = build_nc()
    return _NC


def kernel(context_vector, pond_assignments, tables):
    B, S, D_ = context_vector.shape
    assert D_ == D and B * S == N_CORES * TOK_PER_CORE
    ctx_flat = np.ascontiguousarray(
        np.asarray(context_vector, dtype=np.float32).reshape(B * S, D)
    )
    ponds_flat = np.ascontiguousarray(
        np.asarray(pond_assignments, dtype=np.int32).reshape(B * S)
    )
    tables_flat = np.ascontiguousarray(
        np.asarray(tables, dtype=np.float32).reshape(N_ROWS, D)
    )

    in_maps = [
        {
            "ctx": ctx_flat[c * TOK_PER_CORE:(c + 1) * TOK_PER_CORE],
            "ponds": ponds_flat[c * TOK_PER_CORE:(c + 1) * TOK_PER_CORE],
            "tables": tables_flat,
        }
        for c in range(N_CORES)
    ]

    nc = _get_nc()
    kw = {}
    tc_env = os.environ.get("KERNEL_TRACE_CORES")
    if tc_env:
        kw["trace_cores"] = [int(x) for x in tc_env.split(",")]
    res = bass_utils.run_bass_kernel_spmd(
        nc, in_maps, core_ids=list(range(N_CORES)), **kw
    )
    global LAST_RESULTS
    LAST_RESULTS = res
    out = np.concatenate(
        [np.asarray(res.results[c]["out"]) for c in range(N_CORES)], axis=0
    )
    return out.astype(np.float32).reshape(B, S, D)
